# revision 8
# baseline (speedup 1.0000x reference)
"""Multi-class 3D DICE loss on 8 Trainium2 NeuronCores.

Data-parallel over the subject (batch) axis: core b reduces subject b's
[C=4, 64, 128, 128] volumes to a single per-subject loss scalar; the host
averages the 8 scalars.

Per-core layout: each input tensor is viewed as [128, 32768] where
partition q = c*32 + p (class c in partition block [32c, 32c+32)).
Per chunk of [128, 4096]:
  - DVE  tensor_tensor_reduce: partial sums of output*masks   (inter)
  - ACT  activation(Copy, accum_out): partial sums of masks and output
  - PE   collapses partition blocks / classes at the end via tiny matmuls
"""

import os
import sys
from contextlib import ExitStack

import numpy as np

for _p in ("/opt/trn_rl_repo",):
    if _p not in sys.path and os.path.isdir(_p):
        sys.path.insert(0, _p)

import concourse.bass as bass  # noqa: E402
import concourse.tile as tile  # noqa: E402
from concourse import bacc, mybir  # noqa: E402
from concourse.bass_utils import run_bass_kernel_spmd  # noqa: E402

N_CORES = 8
B, C = 8, 4
SPATIAL = 64 * 128 * 128            # 1,048,576 per (subject, class)
P = 128                             # SBUF partitions = C * 32
COLS = (C * SPATIAL) // P           # 32768 elements per partition
# Descending chunk schedule: big DMAs (4 MiB) for bandwidth in the steady
# state, small chunks at the end so the post-last-byte compute tail is tiny.
CHUNKS = [6144, 6144, 6144, 6144, 6144, 1024, 512, 512]
assert sum(CHUNKS) == COLS
NCHUNK = len(CHUNKS)
FDMAX = max(CHUNKS)
EPS = 1e-7
F32 = mybir.dt.float32
BF16 = mybir.dt.bfloat16


def _dice_body(ctx: ExitStack, tc: "tile.TileContext", out_ap, x_ap, m_ap):
    nc = tc.nc
    add = mybir.AluOpType.add
    mult = mybir.AluOpType.mult
    Copy = mybir.ActivationFunctionType.Copy

    consts = ctx.enter_context(tc.tile_pool(name="consts", bufs=1))
    xpool = ctx.enter_context(tc.tile_pool(name="xin", bufs=3))
    mpool = ctx.enter_context(tc.tile_pool(name="min", bufs=3))
    jpool = ctx.enter_context(tc.tile_pool(name="junk", bufs=1))
    small = ctx.enter_context(tc.tile_pool(name="small", bufs=1))
    psum = ctx.enter_context(tc.tile_pool(name="psum", bufs=1, space="PSUM"))

    # Block indicator: ind[q, c] = 1.0 iff q // 32 == c. lhsT for the
    # partition-block -> per-class collapse.
    ind = consts.tile([P, C], F32)
    nc.vector.memset(ind[:], 0.0)
    for c in range(C):
        nc.vector.memset(ind[c * 32 : (c + 1) * 32, c : c + 1], 1.0)
    ones4 = consts.tile([C, 1], F32)
    nc.vector.memset(ones4[:], 1.0)

    # Per-chunk partial sums (column j <- chunk j); no cross-chunk deps.
    acc_i = small.tile([P, NCHUNK], F32)  # sum(x*m)
    acc_m = small.tile([P, NCHUNK], F32)  # sum(m)
    acc_x = small.tile([P, NCHUNK], F32)  # sum(x)

    off = 0
    for j, fd in enumerate(CHUNKS):
        xt = xpool.tile([P, fd], F32, tag="xt")
        nc.sync.dma_start(out=xt[:], in_=x_ap[:, off : off + fd])
        mt = mpool.tile([P, fd], F32, tag="mt")
        nc.gpsimd.dma_start(out=mt[:], in_=m_ap[:, off : off + fd])
        off += fd

        # inter partials on DVE: out = (x*1)*m, accum = X-reduce(out).
        # Scratch outs are bf16 (never read back) to halve SBUF footprint.
        s0 = jpool.tile([P, fd], BF16, tag="dve_junk")
        nc.vector.scalar_tensor_tensor(
            out=s0[:],
            in0=xt[:],
            scalar=1.0,
            in1=mt[:],
            op0=mult,
            op1=mult,
            accum_out=acc_i[:, j : j + 1],
        )
        s1 = jpool.tile([P, fd], BF16, tag="act_junk")
        nc.scalar.activation(
            out=s1[:], in_=mt[:], func=Copy, accum_out=acc_m[:, j : j + 1]
        )
        s2 = jpool.tile([P, fd], BF16, tag="act_junk")
        nc.scalar.activation(
            out=s2[:], in_=xt[:], func=Copy, accum_out=acc_x[:, j : j + 1]
        )

    # Collapse chunk columns -> [P, 1] each, packed into adjacent columns.
    stacked = small.tile([P, 4], F32)
    nc.vector.tensor_reduce(stacked[:, 0:1], acc_i[:], axis=mybir.AxisListType.X, op=add)
    nc.vector.tensor_reduce(stacked[:, 1:2], acc_m[:], axis=mybir.AxisListType.X, op=add)
    nc.vector.tensor_reduce(stacked[:, 2:3], acc_x[:], axis=mybir.AxisListType.X, op=add)

    # Partition blocks -> per-class sums: [4, 3] = ind.T @ stacked[:, 0:3].
    ps = psum.tile([C, 4], F32)
    nc.tensor.matmul(out=ps[:, 0:3], lhsT=ind[:], rhs=stacked[:, 0:3], start=True, stop=True)

    # sums columns: 0 inter, 1 msum, 2 xsum, 3 total, 4 ms2e, 5 w, 6 wi, 7 wt
    sums = small.tile([C, 8], F32)
    nc.vector.tensor_copy(out=sums[:, 0:3], in_=ps[:, 0:3])
    nc.vector.tensor_add(out=sums[:, 3:4], in0=sums[:, 1:2], in1=sums[:, 2:3])
    nc.vector.tensor_mul(out=sums[:, 4:5], in0=sums[:, 1:2], in1=sums[:, 1:2])
    nc.vector.tensor_scalar_add(out=sums[:, 4:5], in0=sums[:, 4:5], scalar1=EPS)
    nc.vector.reciprocal(out=sums[:, 5:6], in_=sums[:, 4:5])
    nc.vector.tensor_mul(out=sums[:, 6:7], in0=sums[:, 5:6], in1=sums[:, 0:1])
    nc.vector.tensor_mul(out=sums[:, 7:8], in0=sums[:, 5:6], in1=sums[:, 3:4])

    # Class sums: [1, 2] = ones4.T @ [wi, wt]  ->  (nom, sum_c w*total)
    nd_ps = psum.tile([1, 2], F32)
    nc.tensor.matmul(out=nd_ps[:], lhsT=ones4[:], rhs=sums[:, 6:8], start=True, stop=True)

    # fin columns: 0 nom, 1 den_sum, 2 denom, 3 1/denom, 4 nom/denom, 5 result
    fin = small.tile([1, 6], F32)
    nc.vector.tensor_copy(out=fin[:, 0:2], in_=nd_ps[:])
    # denom = sum_c(w*total + EPS) = den_sum + C*EPS
    nc.vector.tensor_scalar_add(out=fin[:, 2:3], in0=fin[:, 1:2], scalar1=C * EPS)
    nc.vector.reciprocal(out=fin[:, 3:4], in_=fin[:, 2:3])
    nc.vector.tensor_mul(out=fin[:, 4:5], in0=fin[:, 0:1], in1=fin[:, 3:4])
    # per-subject loss = 1 - 2 * nom/denom
    nc.vector.tensor_scalar(
        out=fin[:, 5:6], in0=fin[:, 4:5], scalar1=-2.0, scalar2=1.0, op0=mult, op1=add
    )
    nc.sync.dma_start(out=out_ap, in_=fin[:, 5:6])


_CACHE: dict[str, object] = {}


def _build():
    if "nc" in _CACHE:
        return _CACHE["nc"]
    nc = bacc.Bacc("TRN2", target_bir_lowering=False, debug=False)
    x = nc.dram_tensor("x", [P, COLS], F32, kind="ExternalInput").ap()
    m = nc.dram_tensor("m", [P, COLS], F32, kind="ExternalInput").ap()
    out = nc.dram_tensor("loss_partial", [1, 1], F32, kind="ExternalOutput").ap()
    with tile.TileContext(nc) as tc:
        with ExitStack() as ctx:
            _dice_body(ctx, tc, out, x, m)
    nc.compile()
    _CACHE["nc"] = nc
    return nc


def _in_maps(output: np.ndarray, masks: np.ndarray):
    output = np.ascontiguousarray(output, dtype=np.float32)
    masks = np.ascontiguousarray(masks, dtype=np.float32)
    return [
        {"x": output[b].reshape(P, COLS), "m": masks[b].reshape(P, COLS)}
        for b in range(N_CORES)
    ]


def run_sharded(output: np.ndarray, masks: np.ndarray, **spmd_kwargs):
    """Run the SPMD kernel; returns (loss[1], BassKernelResults)."""
    nc = _build()
    res = run_bass_kernel_spmd(
        nc, _in_maps(output, masks), list(range(N_CORES)), **spmd_kwargs
    )
    per_subj = np.array(
        [res.results[b]["loss_partial"][0, 0] for b in range(N_CORES)],
        dtype=np.float32,
    )
    loss = (per_subj.sum(dtype=np.float32) / np.float32(B)).reshape(1)
    return loss.astype(np.float32), res


def kernel(output: np.ndarray, masks: np.ndarray) -> np.ndarray:
    loss, _ = run_sharded(output, masks)
    return loss


# revision 9
# speedup vs baseline: 1.3107x; 1.3107x over previous
"""Multi-class 3D DICE loss on 8 Trainium2 NeuronCores.

Data-parallel over the subject (batch) axis: core b reduces subject b's
[C=4, 64, 128, 128] volumes to a single per-subject loss scalar; the host
averages the 8 scalars.

Per-core layout: each input tensor is viewed as [128, 32768] where
partition q = c*32 + p (class c in partition block [32c, 32c+32)).
Per chunk of [128, 4096]:
  - DVE  tensor_tensor_reduce: partial sums of output*masks   (inter)
  - ACT  activation(Copy, accum_out): partial sums of masks and output
  - PE   collapses partition blocks / classes at the end via tiny matmuls
"""

import os
import sys
from contextlib import ExitStack

import numpy as np

for _p in ("/opt/trn_rl_repo",):
    if _p not in sys.path and os.path.isdir(_p):
        sys.path.insert(0, _p)

import concourse.bass as bass  # noqa: E402
import concourse.tile as tile  # noqa: E402
from concourse import bacc, mybir  # noqa: E402
from concourse.bass_utils import run_bass_kernel_spmd  # noqa: E402

N_CORES = 8
B, C = 8, 4
SPATIAL = 64 * 128 * 128            # 1,048,576 per (subject, class)
P = 128                             # SBUF partitions = C * 32
COLS = (C * SPATIAL) // P           # 32768 elements per partition
# Descending chunk schedule: big DMAs (4 MiB) for bandwidth in the steady
# state, small chunks at the end so the post-last-byte compute tail is tiny.
CHUNKS = [6144, 6144, 6144, 6144, 6144, 1024, 512, 512]
assert sum(CHUNKS) == COLS
NCHUNK = len(CHUNKS)
FDMAX = max(CHUNKS)
EPS = 1e-7
F32 = mybir.dt.float32
BF16 = mybir.dt.bfloat16


def _dice_body(ctx: ExitStack, tc: "tile.TileContext", out_ap, x_ap, m_ap):
    nc = tc.nc
    add = mybir.AluOpType.add
    mult = mybir.AluOpType.mult
    Copy = mybir.ActivationFunctionType.Copy

    consts = ctx.enter_context(tc.tile_pool(name="consts", bufs=1))
    xpool = ctx.enter_context(tc.tile_pool(name="xin", bufs=3))
    mpool = ctx.enter_context(tc.tile_pool(name="min", bufs=3))
    jpool = ctx.enter_context(tc.tile_pool(name="junk", bufs=1))
    small = ctx.enter_context(tc.tile_pool(name="small", bufs=1))
    psum = ctx.enter_context(tc.tile_pool(name="psum", bufs=1, space="PSUM"))

    # Block indicator: ind[q, c] = 1.0 iff q // 32 == c. lhsT for the
    # partition-block -> per-class collapse.
    ind = consts.tile([P, C], F32)
    nc.vector.memset(ind[:], 0.0)
    for c in range(C):
        nc.vector.memset(ind[c * 32 : (c + 1) * 32, c : c + 1], 1.0)
    ones4 = consts.tile([C, 1], F32)
    nc.vector.memset(ones4[:], 1.0)

    # Per-chunk partial sums (column j <- chunk j); no cross-chunk deps.
    acc_i = small.tile([P, NCHUNK], F32)  # sum(x*m)
    acc_m = small.tile([P, NCHUNK], F32)  # sum(m)
    acc_x = small.tile([P, NCHUNK], F32)  # sum(x)

    off = 0
    for j, fd in enumerate(CHUNKS):
        xt = xpool.tile([P, fd], F32, tag="xt")
        nc.sync.dma_start(out=xt[:], in_=x_ap[:, off : off + fd])
        mt = mpool.tile([P, fd], F32, tag="mt")
        nc.sync.dma_start(out=mt[:], in_=m_ap[:, off : off + fd])
        off += fd

        # inter partials on DVE: out = (x*1)*m, accum = X-reduce(out).
        # Scratch outs are bf16 (never read back) to halve SBUF footprint.
        s0 = jpool.tile([P, fd], BF16, tag="dve_junk")
        nc.vector.scalar_tensor_tensor(
            out=s0[:],
            in0=xt[:],
            scalar=1.0,
            in1=mt[:],
            op0=mult,
            op1=mult,
            accum_out=acc_i[:, j : j + 1],
        )
        s1 = jpool.tile([P, fd], BF16, tag="act_junk")
        nc.scalar.activation(
            out=s1[:], in_=mt[:], func=Copy, accum_out=acc_m[:, j : j + 1]
        )
        s2 = jpool.tile([P, fd], BF16, tag="act_junk")
        nc.scalar.activation(
            out=s2[:], in_=xt[:], func=Copy, accum_out=acc_x[:, j : j + 1]
        )

    # Collapse chunk columns -> [P, 1] each, packed into adjacent columns.
    stacked = small.tile([P, 4], F32)
    nc.vector.tensor_reduce(stacked[:, 0:1], acc_i[:], axis=mybir.AxisListType.X, op=add)
    nc.vector.tensor_reduce(stacked[:, 1:2], acc_m[:], axis=mybir.AxisListType.X, op=add)
    nc.vector.tensor_reduce(stacked[:, 2:3], acc_x[:], axis=mybir.AxisListType.X, op=add)

    # Partition blocks -> per-class sums: [4, 3] = ind.T @ stacked[:, 0:3].
    ps = psum.tile([C, 4], F32)
    nc.tensor.matmul(out=ps[:, 0:3], lhsT=ind[:], rhs=stacked[:, 0:3], start=True, stop=True)

    # sums columns: 0 inter, 1 msum, 2 xsum, 3 total, 4 ms2e, 5 w, 6 wi, 7 wt
    sums = small.tile([C, 8], F32)
    nc.vector.tensor_copy(out=sums[:, 0:3], in_=ps[:, 0:3])
    nc.vector.tensor_add(out=sums[:, 3:4], in0=sums[:, 1:2], in1=sums[:, 2:3])
    nc.vector.tensor_mul(out=sums[:, 4:5], in0=sums[:, 1:2], in1=sums[:, 1:2])
    nc.vector.tensor_scalar_add(out=sums[:, 4:5], in0=sums[:, 4:5], scalar1=EPS)
    nc.vector.reciprocal(out=sums[:, 5:6], in_=sums[:, 4:5])
    nc.vector.tensor_mul(out=sums[:, 6:7], in0=sums[:, 5:6], in1=sums[:, 0:1])
    nc.vector.tensor_mul(out=sums[:, 7:8], in0=sums[:, 5:6], in1=sums[:, 3:4])

    # Class sums: [1, 2] = ones4.T @ [wi, wt]  ->  (nom, sum_c w*total)
    nd_ps = psum.tile([1, 2], F32)
    nc.tensor.matmul(out=nd_ps[:], lhsT=ones4[:], rhs=sums[:, 6:8], start=True, stop=True)

    # fin columns: 0 nom, 1 den_sum, 2 denom, 3 1/denom, 4 nom/denom, 5 result
    fin = small.tile([1, 6], F32)
    nc.vector.tensor_copy(out=fin[:, 0:2], in_=nd_ps[:])
    # denom = sum_c(w*total + EPS) = den_sum + C*EPS
    nc.vector.tensor_scalar_add(out=fin[:, 2:3], in0=fin[:, 1:2], scalar1=C * EPS)
    nc.vector.reciprocal(out=fin[:, 3:4], in_=fin[:, 2:3])
    nc.vector.tensor_mul(out=fin[:, 4:5], in0=fin[:, 0:1], in1=fin[:, 3:4])
    # per-subject loss = 1 - 2 * nom/denom
    nc.vector.tensor_scalar(
        out=fin[:, 5:6], in0=fin[:, 4:5], scalar1=-2.0, scalar2=1.0, op0=mult, op1=add
    )
    nc.sync.dma_start(out=out_ap, in_=fin[:, 5:6])


_CACHE: dict[str, object] = {}


def _build():
    if "nc" in _CACHE:
        return _CACHE["nc"]
    nc = bacc.Bacc("TRN2", target_bir_lowering=False, debug=False)
    x = nc.dram_tensor("x", [P, COLS], F32, kind="ExternalInput").ap()
    m = nc.dram_tensor("m", [P, COLS], F32, kind="ExternalInput").ap()
    out = nc.dram_tensor("loss_partial", [1, 1], F32, kind="ExternalOutput").ap()
    with tile.TileContext(nc) as tc:
        with ExitStack() as ctx:
            _dice_body(ctx, tc, out, x, m)
    nc.compile()
    _CACHE["nc"] = nc
    return nc


def _in_maps(output: np.ndarray, masks: np.ndarray):
    output = np.ascontiguousarray(output, dtype=np.float32)
    masks = np.ascontiguousarray(masks, dtype=np.float32)
    return [
        {"x": output[b].reshape(P, COLS), "m": masks[b].reshape(P, COLS)}
        for b in range(N_CORES)
    ]


def run_sharded(output: np.ndarray, masks: np.ndarray, **spmd_kwargs):
    """Run the SPMD kernel; returns (loss[1], BassKernelResults)."""
    nc = _build()
    res = run_bass_kernel_spmd(
        nc, _in_maps(output, masks), list(range(N_CORES)), **spmd_kwargs
    )
    per_subj = np.array(
        [res.results[b]["loss_partial"][0, 0] for b in range(N_CORES)],
        dtype=np.float32,
    )
    loss = (per_subj.sum(dtype=np.float32) / np.float32(B)).reshape(1)
    return loss.astype(np.float32), res


def kernel(output: np.ndarray, masks: np.ndarray) -> np.ndarray:
    loss, _ = run_sharded(output, masks)
    return loss


# revision 15
# speedup vs baseline: 1.3494x; 1.0296x over previous
"""Multi-class 3D DICE loss on 8 Trainium2 NeuronCores.

Data-parallel over the subject (batch) axis: core b reduces subject b's
[C=4, 64, 128, 128] volumes to a single per-subject loss scalar; the host
averages the 8 scalars.

Per-core layout: each input tensor is viewed as [128, 32768] where
partition q = c*32 + p (class c in partition block [32c, 32c+32)).
Per chunk of [128, 4096]:
  - DVE  tensor_tensor_reduce: partial sums of output*masks   (inter)
  - ACT  activation(Copy, accum_out): partial sums of masks and output
  - PE   collapses partition blocks / classes at the end via tiny matmuls
"""

import os
import sys
from contextlib import ExitStack

import numpy as np

for _p in ("/opt/trn_rl_repo",):
    if _p not in sys.path and os.path.isdir(_p):
        sys.path.insert(0, _p)

import concourse.bass as bass  # noqa: E402
import concourse.tile as tile  # noqa: E402
from concourse import bacc, mybir  # noqa: E402
from concourse.bass_utils import run_bass_kernel_spmd  # noqa: E402

N_CORES = 8
B, C = 8, 4
SPATIAL = 64 * 128 * 128            # 1,048,576 per (subject, class)
P = 128                             # SBUF partitions = C * 32
COLS = (C * SPATIAL) // P           # 32768 elements per partition
# Descending chunk schedule: big DMAs (4 MiB) for bandwidth in the steady
# state, small chunks at the end so the post-last-byte compute tail is tiny.
CHUNKS = [4096] * 7 + [2048, 1024, 512, 512]
assert sum(CHUNKS) == COLS
NCHUNK = len(CHUNKS)
FDMAX = max(CHUNKS)
EPS = 1e-7
F32 = mybir.dt.float32
BF16 = mybir.dt.bfloat16


def _dice_body(ctx: ExitStack, tc: "tile.TileContext", out_ap, x_ap, m_ap):
    nc = tc.nc
    add = mybir.AluOpType.add
    mult = mybir.AluOpType.mult
    Copy = mybir.ActivationFunctionType.Copy

    consts = ctx.enter_context(tc.tile_pool(name="consts", bufs=1))
    xpool = ctx.enter_context(tc.tile_pool(name="xin", bufs=5))
    mpool = ctx.enter_context(tc.tile_pool(name="min", bufs=5))
    small = ctx.enter_context(tc.tile_pool(name="small", bufs=1))
    psum = ctx.enter_context(tc.tile_pool(name="psum", bufs=1, space="PSUM"))

    # Block indicator: ind[q, c] = 1.0 iff q // 32 == c. lhsT for the
    # partition-block -> per-class collapse.
    ind = consts.tile([P, C], F32)
    nc.vector.memset(ind[:], 0.0)
    for c in range(C):
        nc.vector.memset(ind[c * 32 : (c + 1) * 32, c : c + 1], 1.0)
    ones4 = consts.tile([C, 1], F32)
    nc.vector.memset(ones4[:], 1.0)

    # Per-chunk partial sums (column j <- chunk j); no cross-chunk deps.
    acc_i = small.tile([P, NCHUNK], F32)  # sum(x*m)
    acc_m = small.tile([P, NCHUNK], F32)  # sum(m)
    acc_x = small.tile([P, NCHUNK], F32)  # sum(x)
    # Engines must write their full elementwise result somewhere; stride-0
    # broadcast dummies avoid real [P, fd] scratch tiles (HW-verified).
    dve_dummy = small.tile([P, 1], F32)
    act_dummy = small.tile([P, 1], F32)

    off = 0
    for j, fd in enumerate(CHUNKS):
        xt = xpool.tile([P, fd], F32, tag="xt")
        nc.sync.dma_start(out=xt[:], in_=x_ap[:, off : off + fd])
        mt = mpool.tile([P, fd], F32, tag="mt")
        nc.sync.dma_start(out=mt[:], in_=m_ap[:, off : off + fd])
        off += fd

        # inter partials on DVE: out = (x*1)*m, accum = X-reduce(out).
        nc.vector.scalar_tensor_tensor(
            out=dve_dummy.broadcast_to((P, fd)),
            in0=xt[:],
            scalar=1.0,
            in1=mt[:],
            op0=mult,
            op1=mult,
            accum_out=acc_i[:, j : j + 1],
        )
        nc.scalar.activation(
            out=act_dummy.broadcast_to((P, fd)),
            in_=mt[:],
            func=Copy,
            accum_out=acc_m[:, j : j + 1],
        )
        nc.scalar.activation(
            out=act_dummy.broadcast_to((P, fd)),
            in_=xt[:],
            func=Copy,
            accum_out=acc_x[:, j : j + 1],
        )

    # Collapse chunk columns -> [P, 1] each, packed into adjacent columns.
    stacked = small.tile([P, 4], F32)
    nc.vector.tensor_reduce(stacked[:, 0:1], acc_i[:], axis=mybir.AxisListType.X, op=add)
    nc.vector.tensor_reduce(stacked[:, 1:2], acc_m[:], axis=mybir.AxisListType.X, op=add)
    nc.vector.tensor_reduce(stacked[:, 2:3], acc_x[:], axis=mybir.AxisListType.X, op=add)

    # Partition blocks -> per-class sums: [4, 3] = ind.T @ stacked[:, 0:3].
    ps = psum.tile([C, 4], F32)
    nc.tensor.matmul(out=ps[:, 0:3], lhsT=ind[:], rhs=stacked[:, 0:3], start=True, stop=True)

    # sums columns: 0 inter, 1 msum, 2 xsum, 3 total, 4 ms2e, 5 w, 6 wi, 7 wt
    sums = small.tile([C, 8], F32)
    nc.vector.tensor_copy(out=sums[:, 0:3], in_=ps[:, 0:3])
    nc.vector.tensor_add(out=sums[:, 3:4], in0=sums[:, 1:2], in1=sums[:, 2:3])
    nc.vector.tensor_mul(out=sums[:, 4:5], in0=sums[:, 1:2], in1=sums[:, 1:2])
    nc.vector.tensor_scalar_add(out=sums[:, 4:5], in0=sums[:, 4:5], scalar1=EPS)
    nc.vector.reciprocal(out=sums[:, 5:6], in_=sums[:, 4:5])
    nc.vector.tensor_mul(out=sums[:, 6:7], in0=sums[:, 5:6], in1=sums[:, 0:1])
    nc.vector.tensor_mul(out=sums[:, 7:8], in0=sums[:, 5:6], in1=sums[:, 3:4])

    # Class sums: [1, 2] = ones4.T @ [wi, wt]  ->  (nom, sum_c w*total)
    nd_ps = psum.tile([1, 2], F32)
    nc.tensor.matmul(out=nd_ps[:], lhsT=ones4[:], rhs=sums[:, 6:8], start=True, stop=True)

    # fin columns: 0 nom, 1 den_sum, 2 denom, 3 1/denom, 4 nom/denom, 5 result
    fin = small.tile([1, 6], F32)
    nc.vector.tensor_copy(out=fin[:, 0:2], in_=nd_ps[:])
    # denom = sum_c(w*total + EPS) = den_sum + C*EPS
    nc.vector.tensor_scalar_add(out=fin[:, 2:3], in0=fin[:, 1:2], scalar1=C * EPS)
    nc.vector.reciprocal(out=fin[:, 3:4], in_=fin[:, 2:3])
    nc.vector.tensor_mul(out=fin[:, 4:5], in0=fin[:, 0:1], in1=fin[:, 3:4])
    # per-subject loss = 1 - 2 * nom/denom
    nc.vector.tensor_scalar(
        out=fin[:, 5:6], in0=fin[:, 4:5], scalar1=-2.0, scalar2=1.0, op0=mult, op1=add
    )
    nc.sync.dma_start(out=out_ap, in_=fin[:, 5:6])


_CACHE: dict[str, object] = {}


def _build():
    if "nc" in _CACHE:
        return _CACHE["nc"]
    nc = bacc.Bacc("TRN2", target_bir_lowering=False, debug=False)
    x = nc.dram_tensor("x", [P, COLS], F32, kind="ExternalInput").ap()
    m = nc.dram_tensor("m", [P, COLS], F32, kind="ExternalInput").ap()
    out = nc.dram_tensor("loss_partial", [1, 1], F32, kind="ExternalOutput").ap()
    with tile.TileContext(nc) as tc:
        with ExitStack() as ctx:
            _dice_body(ctx, tc, out, x, m)
    nc.compile()
    _CACHE["nc"] = nc
    return nc


def _in_maps(output: np.ndarray, masks: np.ndarray):
    output = np.ascontiguousarray(output, dtype=np.float32)
    masks = np.ascontiguousarray(masks, dtype=np.float32)
    return [
        {"x": output[b].reshape(P, COLS), "m": masks[b].reshape(P, COLS)}
        for b in range(N_CORES)
    ]


def run_sharded(output: np.ndarray, masks: np.ndarray, **spmd_kwargs):
    """Run the SPMD kernel; returns (loss[1], BassKernelResults)."""
    nc = _build()
    res = run_bass_kernel_spmd(
        nc, _in_maps(output, masks), list(range(N_CORES)), **spmd_kwargs
    )
    per_subj = np.array(
        [res.results[b]["loss_partial"][0, 0] for b in range(N_CORES)],
        dtype=np.float32,
    )
    loss = (per_subj.sum(dtype=np.float32) / np.float32(B)).reshape(1)
    return loss.astype(np.float32), res


def kernel(output: np.ndarray, masks: np.ndarray) -> np.ndarray:
    loss, _ = run_sharded(output, masks)
    return loss


# revision 18
# speedup vs baseline: 1.5646x; 1.1595x over previous
"""Multi-class 3D DICE loss on 8 Trainium2 NeuronCores.

Data-parallel over the subject (batch) axis: core b reduces subject b's
[C=4, 64, 128, 128] volumes to a single per-subject loss scalar; the host
averages the 8 scalars.

Per-core layout: each input tensor is viewed as [128, 32768] where
partition q = c*32 + p (class c in partition block [32c, 32c+32)).
Per chunk of [128, 4096]:
  - DVE  tensor_tensor_reduce: partial sums of output*masks   (inter)
  - ACT  activation(Copy, accum_out): partial sums of masks and output
  - PE   collapses partition blocks / classes at the end via tiny matmuls
"""

import os
import sys
from contextlib import ExitStack

import numpy as np

for _p in ("/opt/trn_rl_repo",):
    if _p not in sys.path and os.path.isdir(_p):
        sys.path.insert(0, _p)

import concourse.bass as bass  # noqa: E402
import concourse.tile as tile  # noqa: E402
from concourse import bacc, mybir  # noqa: E402
from concourse.bass_utils import run_bass_kernel_spmd  # noqa: E402

N_CORES = 8
B, C = 8, 4
SPATIAL = 64 * 128 * 128            # 1,048,576 per (subject, class)
P = 128                             # SBUF partitions = C * 32
COLS = (C * SPATIAL) // P           # 32768 elements per partition
# Descending chunk schedule: big DMAs (4 MiB) for bandwidth in the steady
# state, small chunks at the end so the post-last-byte compute tail is tiny.
CHUNKS = [8192, 8192, 8192, 4096, 2048, 1024, 512, 512]
N_BIG = 4  # first chunks land in the big pools; the rest in small tail pools
assert sum(CHUNKS) == COLS
NCHUNK = len(CHUNKS)
FDMAX = max(CHUNKS)
EPS = 1e-7
F32 = mybir.dt.float32
BF16 = mybir.dt.bfloat16


def _dice_body(ctx: ExitStack, tc: "tile.TileContext", out_ap, x_ap, m_ap):
    nc = tc.nc
    add = mybir.AluOpType.add
    mult = mybir.AluOpType.mult
    Copy = mybir.ActivationFunctionType.Copy

    consts = ctx.enter_context(tc.tile_pool(name="consts", bufs=1))
    xpool = ctx.enter_context(tc.tile_pool(name="xin", bufs=2))
    mpool = ctx.enter_context(tc.tile_pool(name="min", bufs=2))
    xtail = ctx.enter_context(tc.tile_pool(name="xtail", bufs=3))
    mtail = ctx.enter_context(tc.tile_pool(name="mtail", bufs=3))
    small = ctx.enter_context(tc.tile_pool(name="small", bufs=1))
    psum = ctx.enter_context(tc.tile_pool(name="psum", bufs=1, space="PSUM"))

    # Block indicator: ind[q, c] = 1.0 iff q // 32 == c. lhsT for the
    # partition-block -> per-class collapse.
    ind = consts.tile([P, C], F32)
    nc.vector.memset(ind[:], 0.0)
    for c in range(C):
        nc.vector.memset(ind[c * 32 : (c + 1) * 32, c : c + 1], 1.0)
    ones4 = consts.tile([C, 1], F32)
    nc.vector.memset(ones4[:], 1.0)

    # Per-chunk partial sums (column j <- chunk j); no cross-chunk deps.
    acc_i = small.tile([P, NCHUNK], F32)  # sum(x*m)
    acc_m = small.tile([P, NCHUNK], F32)  # sum(m)
    acc_x = small.tile([P, NCHUNK], F32)  # sum(x)
    # Engines must write their full elementwise result somewhere; stride-0
    # broadcast dummies avoid real [P, fd] scratch tiles (HW-verified).
    dve_dummy = small.tile([P, 1], F32)
    act_dummy = small.tile([P, 1], F32)

    off = 0
    for j, fd in enumerate(CHUNKS):
        xp, mp = (xpool, mpool) if j < N_BIG else (xtail, mtail)
        xt = xp.tile([P, fd], F32, tag="xt")
        nc.sync.dma_start(out=xt[:], in_=x_ap[:, off : off + fd])
        mt = mp.tile([P, fd], F32, tag="mt")
        nc.sync.dma_start(out=mt[:], in_=m_ap[:, off : off + fd])
        off += fd

        # inter partials on DVE: out = (x*1)*m, accum = X-reduce(out).
        nc.vector.scalar_tensor_tensor(
            out=dve_dummy.broadcast_to((P, fd)),
            in0=xt[:],
            scalar=1.0,
            in1=mt[:],
            op0=mult,
            op1=mult,
            accum_out=acc_i[:, j : j + 1],
        )
        nc.scalar.activation(
            out=act_dummy.broadcast_to((P, fd)),
            in_=mt[:],
            func=Copy,
            accum_out=acc_m[:, j : j + 1],
        )
        # x-sum alternates between ACT and DVE so neither engine backlogs.
        if j % 2 == 0:
            nc.scalar.activation(
                out=act_dummy.broadcast_to((P, fd)),
                in_=xt[:],
                func=Copy,
                accum_out=acc_x[:, j : j + 1],
            )
        else:
            nc.vector.tensor_reduce(
                acc_x[:, j : j + 1], xt[:], axis=mybir.AxisListType.X, op=add
            )

    # Collapse chunk columns -> [P, 1] each, packed into adjacent columns.
    stacked = small.tile([P, 4], F32)
    nc.vector.tensor_reduce(stacked[:, 0:1], acc_i[:], axis=mybir.AxisListType.X, op=add)
    nc.vector.tensor_reduce(stacked[:, 1:2], acc_m[:], axis=mybir.AxisListType.X, op=add)
    nc.vector.tensor_reduce(stacked[:, 2:3], acc_x[:], axis=mybir.AxisListType.X, op=add)

    # Partition blocks -> per-class sums: [4, 3] = ind.T @ stacked[:, 0:3].
    ps = psum.tile([C, 4], F32)
    nc.tensor.matmul(out=ps[:, 0:3], lhsT=ind[:], rhs=stacked[:, 0:3], start=True, stop=True)

    # sums columns: 0 inter, 1 msum, 2 xsum, 3 total, 4 ms2e, 5 w, 6 wi, 7 wt
    sums = small.tile([C, 8], F32)
    nc.vector.tensor_copy(out=sums[:, 0:3], in_=ps[:, 0:3])
    nc.vector.tensor_add(out=sums[:, 3:4], in0=sums[:, 1:2], in1=sums[:, 2:3])
    nc.vector.tensor_mul(out=sums[:, 4:5], in0=sums[:, 1:2], in1=sums[:, 1:2])
    nc.vector.tensor_scalar_add(out=sums[:, 4:5], in0=sums[:, 4:5], scalar1=EPS)
    nc.vector.reciprocal(out=sums[:, 5:6], in_=sums[:, 4:5])
    nc.vector.tensor_mul(out=sums[:, 6:7], in0=sums[:, 5:6], in1=sums[:, 0:1])
    nc.vector.tensor_mul(out=sums[:, 7:8], in0=sums[:, 5:6], in1=sums[:, 3:4])

    # Class sums: [1, 2] = ones4.T @ [wi, wt]  ->  (nom, sum_c w*total)
    nd_ps = psum.tile([1, 2], F32)
    nc.tensor.matmul(out=nd_ps[:], lhsT=ones4[:], rhs=sums[:, 6:8], start=True, stop=True)

    # fin columns: 0 nom, 1 den_sum, 2 denom, 3 1/denom, 4 nom/denom, 5 result
    fin = small.tile([1, 6], F32)
    nc.vector.tensor_copy(out=fin[:, 0:2], in_=nd_ps[:])
    # denom = sum_c(w*total + EPS) = den_sum + C*EPS
    nc.vector.tensor_scalar_add(out=fin[:, 2:3], in0=fin[:, 1:2], scalar1=C * EPS)
    nc.vector.reciprocal(out=fin[:, 3:4], in_=fin[:, 2:3])
    nc.vector.tensor_mul(out=fin[:, 4:5], in0=fin[:, 0:1], in1=fin[:, 3:4])
    # per-subject loss = 1 - 2 * nom/denom
    nc.vector.tensor_scalar(
        out=fin[:, 5:6], in0=fin[:, 4:5], scalar1=-2.0, scalar2=1.0, op0=mult, op1=add
    )
    nc.sync.dma_start(out=out_ap, in_=fin[:, 5:6])


_CACHE: dict[str, object] = {}


def _build():
    if "nc" in _CACHE:
        return _CACHE["nc"]
    nc = bacc.Bacc("TRN2", target_bir_lowering=False, debug=False)
    x = nc.dram_tensor("x", [P, COLS], F32, kind="ExternalInput").ap()
    m = nc.dram_tensor("m", [P, COLS], F32, kind="ExternalInput").ap()
    out = nc.dram_tensor("loss_partial", [1, 1], F32, kind="ExternalOutput").ap()
    with tile.TileContext(nc) as tc:
        with ExitStack() as ctx:
            _dice_body(ctx, tc, out, x, m)
    nc.compile()
    _CACHE["nc"] = nc
    return nc


def _in_maps(output: np.ndarray, masks: np.ndarray):
    output = np.ascontiguousarray(output, dtype=np.float32)
    masks = np.ascontiguousarray(masks, dtype=np.float32)
    return [
        {"x": output[b].reshape(P, COLS), "m": masks[b].reshape(P, COLS)}
        for b in range(N_CORES)
    ]


def run_sharded(output: np.ndarray, masks: np.ndarray, **spmd_kwargs):
    """Run the SPMD kernel; returns (loss[1], BassKernelResults)."""
    nc = _build()
    res = run_bass_kernel_spmd(
        nc, _in_maps(output, masks), list(range(N_CORES)), **spmd_kwargs
    )
    per_subj = np.array(
        [res.results[b]["loss_partial"][0, 0] for b in range(N_CORES)],
        dtype=np.float32,
    )
    loss = (per_subj.sum(dtype=np.float32) / np.float32(B)).reshape(1)
    return loss.astype(np.float32), res


def kernel(output: np.ndarray, masks: np.ndarray) -> np.ndarray:
    loss, _ = run_sharded(output, masks)
    return loss
